# revision 49
# baseline (speedup 1.0000x reference)
"""GNN message-passing attention kernel for Trainium2 (Bass/Tile).

Problem: 3 iterations of masked single-head attention over 1024 independent
graphs (N=256 nodes, V=40 features, QK=50), data-parallel on the leading F
axis across 8 NeuronCores (128 graphs/core), full inputs in / full output out.

The axon tunnel to the devices moves ~64 MB/s up / ~30 MB/s down with
~100 ms sync latency, so end-to-end time is dominated by host<->device
bytes and round trips, not device compute (~1 ms/core).  This version
minimizes wire traffic and pipelines it:
  - values cross the wire as int8 with per-row abs-max scales (10.5 MB +
    0.5 MB f16 scales) and are dequantized on-device; the ones-column used
    to fold the q/k biases into the matmuls is memset on-device.
  - adjacency crosses bit-packed (8.4 MB, the entropy floor for random 0/1)
    and is unpacked on the DVE: a broadcast-AP bitwise_and against a
    per-column bitmask, then is_gt(0) -> exact {0,1} bf16.  Host packing is
    a BLAS matvec over the exact 0.0/1.0 floats (np.packbits is GIL-bound).
  - the additive softmax mask is applied by PE matmuls with the unpacked
    adjacency as the *stationary* operand and a MASKC-scaled identity
    streaming, which wants adj[j,l] in its natural row-major layout -- no
    host-side transpose at all.  (MASKC rounds to 7072 in bf16; the +0.13
    shift after /sqrt(50) is uniform across unmasked entries of a row and
    cancels in softmax.)
  - the output is normalized then quantized on-device to uint8 with per-row
    abs-max scales: trunc(x*127/rowmax + 128.5) is exact round-to-nearest
    through the engine's trunc-toward-zero f32->u8 conversion, and the f16
    scale is embedded in the same row via a bitcast view (one 11 MB fetch).
  - all per-graph wire data rides ONE combined uint8 array per direction
    ([*, N, 74] in: int8 values | f16 scale | packed adj; [*, N, 42] out),
    one put + one fetch per segment.
  - the batch is split into SEG=8 segments pipelined over the full-duplex
    tunnel: host encode (fused numba loops -- the single CPU is shared with
    the axon relay) paces uploads so 2-3 stay in flight (the transport's
    sweet spot: deep async queues anti-scale to 18 MB/s, ~2 concurrent
    transfers sustain ~59 MB/s); execs dispatch immediately and wait for
    input device-side; fetches stream back through a 2-deep pool overlapping
    the remaining uploads; donated output buffers (jnp.zeros, device-side)
    for the next call are pre-created after the wire drains; the bitmask
    constant lives on-device across calls.
  - rel err vs the fp32 reference is ~1.1e-2 (int8 in + u8 out quantization,
    deterministic), within the 2e-2 gate.

Dataflow on-device (inherited from the previous version): "transposed-e"
layout, gb=2 graphs per pipeline step, 8 streams phase-interleaved so every
engine always has independent work queued.  e^T[l,j] = k_l . q_j accumulated
on top of the PE-written mask; one Exp ACT per pair produces num^T directly
in the layout the nv matmul wants; per-partition reciprocal + tensor_scalar
normalize during the PSUM->SBUF move, with rowsum*recip == 1.0 refreshing
the ones-column for the next iteration for free.
"""

import math
import sys
from concurrent.futures import ThreadPoolExecutor

import numpy as np

sys.path.insert(0, "/opt/trn_rl_repo")

import concourse.bass as bass  # noqa: E402,F401
import concourse.mybir as mybir  # noqa: E402
from concourse import bacc, bass2jax, tile  # noqa: E402
from concourse.masks import make_identity  # noqa: E402

# Problem constants (hardcoded per harness contract).
F, N, V, QK = 1024, 256, 40, 50
ITERS = 3
SCALE = math.sqrt(50.0)  # NUM_QK = 50
MASKC = 1000.0 * SCALE  # adj * MASKC accumulated into e; exp bias -1000
N_CORES = 8
SEG = 8  # upload/exec/download pipeline segments (the tunnel is full-duplex)
FS = F // SEG  # graphs per segment
G = FS // N_CORES  # graphs per core per segment
NC2 = N // 128  # 2 partition chunks of the node axis
NB = N // 8  # packed adjacency bytes per row
W = V + 2 + NB  # combined wire row: int8 values | f16 scale bytes | packed adj

F32 = mybir.dt.float32
F32R = mybir.dt.float32r  # fp32 data through the fast (replicated) PE path
BF16 = mybir.dt.bfloat16
F16 = mybir.dt.float16
U8 = mybir.dt.uint8
I8 = mybir.dt.int8

DEFAULT_BUFS = dict(
    io=10, work=10, small=11, vnb=22, vhb=8, adjpb=8, andb=8, vob=10,
    pmain=3, paux=2,
)


def build_nc(g_count=G, gb=2, streams=8, group=4, bufs=None):
    """Build the single-core Bass program (SPMD across 8 cores)."""
    B = dict(DEFAULT_BUFS)
    if bufs:
        B.update(bufs)
    streams = min(streams, g_count // gb)
    assert g_count % (gb * streams) == 0
    group = min(group, streams)
    nc = bacc.Bacc("TRN2", target_bir_lowering=False, debug=False)

    comb_d = nc.dram_tensor("comb", [g_count, N, W], U8, kind="ExternalInput")
    wqk_d = nc.dram_tensor("wqk_aug", [2 * (V + 1), QK], F32R, kind="ExternalInput")
    bitm_d = nc.dram_tensor("bitm", [128, N], U8, kind="ExternalInput")
    outc_d = nc.dram_tensor("outc", [g_count, N, V + 2], U8, kind="ExternalOutput")

    with tile.TileContext(nc) as tc:
        with (
            tc.tile_pool(name="const", bufs=1) as constp,
            tc.tile_pool(name="io", bufs=B["io"]) as iop,
            tc.tile_pool(name="work", bufs=B["work"]) as workp,
            tc.tile_pool(name="small", bufs=B["small"]) as smallp,
            tc.tile_pool(name="pmain", bufs=B["pmain"], space="PSUM") as pmainp,
            tc.tile_pool(name="paux", bufs=B["paux"], space="PSUM") as pauxp,
        ):
            wq_sb = constp.tile([V + 1, QK], F32R)
            nc.sync.dma_start(wq_sb, wqk_d[0 : V + 1, :])
            wk_sb = constp.tile([V + 1, QK], F32R)
            nc.sync.dma_start(wk_sb, wqk_d[V + 1 : 2 * (V + 1), :])
            bitm_sb = constp.tile([128, N], U8)
            nc.sync.dma_start(bitm_sb, bitm_d[:, :])
            expbias_sb = constp.tile([128, 1], F32)
            nc.gpsimd.memset(expbias_sb, -1000.0)
            id_f32 = constp.tile([128, 128], F32)
            make_identity(nc, id_f32)
            # MASKC-scaled identity: streamed against stationary adjacency
            # chunks to accumulate the additive mask into PSUM on PE.
            idm_sc = constp.tile([128, 128], BF16)
            nc.vector.tensor_scalar_mul(idm_sc, id_f32, MASKC)

            class Stream:
                pass

            def phase_load(st, g0):
                st.prev_g0 = getattr(st, "g0", None)
                st.prev_vo = getattr(st, "vo", None)
                st.g0 = g0
                gsl = slice(g0, g0 + gb)
                st.comb = iop.tile([128, gb, NC2, W], U8, tag="comb", bufs=B["adjpb"])
                nc.sync.dma_start(
                    st.comb, comb_d[gsl, :, :].rearrange("g (c p) w -> p g c w", c=NC2)
                )

            def phase_prep(st):
                # int8 -> fp32 dequant by the per-row scale (already /127 on
                # host); the ones-column rides the same tile so the q/k
                # biases stay inside the weight matmuls.
                vsc = smallp.tile([128, gb, NC2], F32, tag="vsc")
                nc.vector.tensor_copy(
                    vsc.unsqueeze(-1), st.comb[:, :, :, V : V + 2].bitcast(F16)
                )
                st.vn = iop.tile([128, gb, NC2, V + 1], F32, tag="vn", bufs=B["vnb"])
                for g in range(gb):
                    for c in range(NC2):
                        nc.vector.tensor_scalar_mul(
                            st.vn[:, g, c, 0:V],
                            st.comb[:, g, c, 0:V].bitcast(I8),
                            vsc[:, g, c : c + 1],
                        )
                nc.gpsimd.memset(st.vn[:, :, :, V], 1.0)
                # unpack adjacency bits: (byte & bitmask) > 0 -> {0,1} bf16,
                # laid out adj[j-part, l-free] for stationary mask matmuls.
                t_and = smallp.tile([128, gb, NC2, N], U8, tag="andt", bufs=B["andb"])
                src = (
                    st.comb[:, :, :, V + 2 : W]
                    .rearrange("p g c b -> p (g c) b")
                    .unsqueeze(-1)
                    .broadcast_to([128, gb * NC2, NB, 8])
                )
                msk = (
                    bitm_sb[:, :]
                    .rearrange("p (b e) -> p b e", e=8)
                    .unsqueeze(1)
                    .broadcast_to([128, gb * NC2, NB, 8])
                )
                dst = t_and[:, :, :, :].rearrange("p g c (b e) -> p (g c) b e", e=8)
                nc.vector.tensor_tensor(dst, src, msk, op=mybir.AluOpType.bitwise_and)
                st.adj = iop.tile([128, gb, NC2, N], BF16, tag="adj")
                nc.vector.tensor_single_scalar(
                    st.adj, t_and, 0, op=mybir.AluOpType.is_gt
                )
                st.comb = None

            def phase_vt0(st):
                psum_vt = pauxp.tile([V + 1, gb * N], F32, tag="paux")
                for g in range(gb):
                    for c in range(NC2):
                        nc.tensor.transpose(
                            psum_vt[:, N * g + 128 * c : N * g + 128 * (c + 1)],
                            st.vn[:, g, c, :],
                            id_f32,
                        )
                st.vt = smallp.tile([V + 1, gb * N], F32R, tag="vt")
                nc.vector.tensor_copy(st.vt, psum_vt)

            def phase_qk(st):
                # [50, (qk-half, g, j)]: q in bank 0, k in bank 1.
                # Bias rides the vt ones-row (weights row V).
                st.psum_qk = pmainp.tile([QK, 2 * gb * N], F32, tag="pmain")
                nc.tensor.matmul(st.psum_qk[:, 0 : gb * N], wq_sb, st.vt)
                nc.tensor.matmul(st.psum_qk[:, gb * N : 2 * gb * N], wk_sb, st.vt)

            def phase_tanh(st):
                st.qk = workp.tile([QK, 2 * gb * N], F32R, tag="qk")
                nc.scalar.activation(
                    st.qk, st.psum_qk, mybir.ActivationFunctionType.Tanh
                )
                st.psum_qk = None

            def phase_mask(st):
                # additive mask preloaded into PSUM on PE: stationary
                # adjacency chunk [j-part, l-free], streaming MASKC-scaled
                # identity -> psum_e[l, j] = MASKC * adj[j, l].
                st.psum_e = pmainp.tile([128, gb, NC2 * N], F32, tag="pmain", name="pe")
                # each graph's e-block is one 2KB PSUM zero region; start=True
                # (which re-marks the whole region pending-zero) only on the
                # first of its four chunk matmuls -- the rest land on
                # still-pending bytes and overwrite their own chunk.
                for g in range(gb):
                    for lc in range(NC2):
                        for jc in range(NC2):
                            nc.tensor.matmul(
                                st.psum_e[
                                    :, g, N * lc + 128 * jc : N * lc + 128 * (jc + 1)
                                ],
                                st.adj[:, g, jc, 128 * lc : 128 * (lc + 1)],
                                idm_sc,
                                start=(lc == 0 and jc == 0),
                                stop=False,
                                skip_group_check=True,
                            )

            def phase_et(st):
                for g in range(gb):
                    for lc in range(NC2):
                        nc.tensor.matmul(
                            st.psum_e[:, g, N * lc : N * (lc + 1)],
                            st.qk[:, gb * N + N * g + 128 * lc : gb * N + N * g + 128 * (lc + 1)],
                            st.qk[:, N * g : N * (g + 1)],
                            start=False,
                            stop=True,
                            skip_group_check=True,
                        )

            def phase_exp(st):
                st.numt = workp.tile([128, gb, NC2 * N], F32, tag="numt")
                nc.scalar.activation(
                    st.numt,
                    st.psum_e,
                    mybir.ActivationFunctionType.Exp,
                    bias=expbias_sb,
                    scale=1.0 / SCALE,
                )
                st.psum_e = None

            def phase_nv(st):
                # nv[j, v] = sum_l num[j, l] v[l, v], directly off numT
                # (l already on partitions); the vn ones-column makes col V
                # the softmax row-sum.
                st.psum_nv = pauxp.tile([128, gb, NC2, V + 1], F32, tag="paux")
                for g in range(gb):
                    for jc in range(NC2):
                        for lc in range(NC2):
                            nc.tensor.matmul(
                                st.psum_nv[:, g, jc, :],
                                st.numt[:, g, N * lc + 128 * jc : N * lc + 128 * jc + 128],
                                st.vn[:, g, lc, :],
                                start=(lc == 0),
                                stop=(lc == NC2 - 1),
                            )
                st.numt = None

            def phase_norm(st, last):
                recip = smallp.tile([128, gb, NC2], F32, tag="recip")
                nc.vector.reciprocal(recip, st.psum_nv[:, :, :, V])
                if last:
                    # final iteration: normalize, then quantize each row to
                    # uint8 with its own abs-max scale.  trunc(x*127/rowmax
                    # + 128.5) is exact round-to-nearest (everything
                    # positive, so the engine's trunc-toward-zero == floor;
                    # max lands on 255.5-eps, no wrap); host decodes as
                    # (k - 128) * (rowmax/127) from the shipped fp16 scale.
                    vo32 = workp.tile([128, gb, NC2, V], F32, tag="vo32")
                    rowmax = smallp.tile([128, gb, NC2], F32, tag="rowmax")
                    for g in range(gb):
                        for jc in range(NC2):
                            nc.vector.tensor_scalar_mul(
                                vo32[:, g, jc, :],
                                st.psum_nv[:, g, jc, 0:V],
                                recip[:, g, jc : jc + 1],
                            )
                            nc.vector.tensor_reduce(
                                rowmax[:, g, jc : jc + 1],
                                vo32[:, g, jc, :],
                                axis=mybir.AxisListType.X,
                                op=mybir.AluOpType.max,
                                apply_absolute_value=True,
                            )
                    qs = smallp.tile([128, gb, NC2], F32, tag="qs")
                    nc.vector.reciprocal(qs, rowmax)
                    qs127 = smallp.tile([128, gb, NC2], F32, tag="qs127")
                    nc.vector.tensor_scalar_mul(qs127, qs, 127.0)
                    st.vo = workp.tile(
                        [128, gb, NC2, V + 2], U8, tag="vo", bufs=B["vob"]
                    )
                    nc.vector.tensor_scalar_mul(
                        st.vo[:, :, :, V : V + 2].bitcast(F16),
                        rowmax.unsqueeze(-1),
                        1.0 / 127.0,
                    )
                    for g in range(gb):
                        for jc in range(NC2):
                            nc.vector.tensor_scalar(
                                st.vo[:, g, jc, 0:V],
                                vo32[:, g, jc, :],
                                qs127[:, g, jc : jc + 1],
                                128.5,
                                op0=mybir.AluOpType.mult,
                                op1=mybir.AluOpType.add,
                            )
                else:
                    st.vn = iop.tile([128, gb, NC2, V + 1], F32, tag="vn", bufs=B["vnb"])
                    for g in range(gb):
                        for jc in range(NC2):
                            nc.vector.tensor_scalar_mul(
                                st.vn[:, g, jc, :],
                                st.psum_nv[:, g, jc, :],
                                recip[:, g, jc : jc + 1],
                            )
                st.psum_nv = None

            def phase_vt(st):
                psum_vt = pauxp.tile([V + 1, gb * N], F32, tag="paux")
                for g in range(gb):
                    for jc in range(NC2):
                        nc.tensor.transpose(
                            psum_vt[:, N * g + 128 * jc : N * g + 128 * (jc + 1)],
                            st.vn[:, g, jc, :],
                            id_f32,
                        )
                st.vt = smallp.tile([V + 1, gb * N], F32R, tag="vt")
                nc.vector.tensor_copy(st.vt, psum_vt)

            def phase_store_prev(st):
                # SWDGE (gpsimd) queue: keeps result stores out of the SP
                # FIFO so the next round's loads always prefetch early.
                gsl = slice(st.prev_g0, st.prev_g0 + gb)
                nc.gpsimd.dma_start(
                    outc_d[gsl, :, :].rearrange("g (c p) v -> p g c v", c=NC2),
                    st.prev_vo,
                )

            sts = [Stream() for _ in range(streams)]
            for _i, _st in enumerate(sts):
                _st.sid = _i
            grps = [sts[i : i + group] for i in range(0, streams, group)]

            def run_iter(grp, t):
                for st in grp:
                    phase_qk(st)
                for st in grp:
                    phase_mask(st)
                for st in grp:
                    phase_tanh(st)
                for st in grp:
                    phase_et(st)
                for st in grp:
                    phase_exp(st)
                for st in grp:
                    phase_nv(st)
                for st in grp:
                    phase_norm(st, t == ITERS - 1)
                if t < ITERS - 1:
                    for st in grp:
                        phase_vt(st)

            # Groups round-robin per iteration so one group's next phase
            # fills the pipeline while the other finishes; the previous
            # round's store and the next round's load ride inside the
            # rotation so round boundaries never resynchronize the streams.
            rounds = g_count // (gb * streams)
            for r in range(rounds):
                for grp in grps:
                    for st in grp:
                        phase_load(st, gb * (r * streams + st.sid))
                for grp in grps:
                    for st in grp:
                        if r > 0:
                            phase_store_prev(st)
                    for st in grp:
                        phase_prep(st)
                    for st in grp:
                        phase_vt0(st)
                for t in range(ITERS):
                    for grp in grps:
                        run_iter(grp, t)
            for grp in grps:
                for st in grp:
                    st.prev_g0, st.prev_vo = st.g0, st.vo
                    phase_store_prev(st)

    nc.compile()
    return nc


# ---------------------------------------------------------------------------
# Execution path: cached jitted shard_map over 8 cores, bypassing
# run_bass_via_pjrt's host-side concats / host-zero donation buffers.
# ---------------------------------------------------------------------------

_IO_POOL = ThreadPoolExecutor(32)  # wire gets + zeros (block, don't compute)
# 2-deep fetch pool: bounded downlink concurrency avoids the deep-queue
# anti-scaling the transport shows when many transfers are registered at once
_GET_POOL = ThreadPoolExecutor(2)


class _Exec:
    pass


_EXEC = None


def _build_exec():
    import jax
    import jax.numpy as jnp
    from jax.experimental.shard_map import shard_map
    from jax.sharding import Mesh, NamedSharding, PartitionSpec

    nc = build_nc()
    bass2jax.install_neuronx_cc_hook()
    assert nc.dbg_addr is None
    partition_name = nc.partition_id_tensor.name if nc.partition_id_tensor else None

    in_names, out_names, out_avals = [], [], []
    for alloc in nc.m.functions[0].allocations:
        if not isinstance(alloc, mybir.MemoryLocationSet):
            continue
        name = alloc.memorylocations[0].name
        if alloc.kind == "ExternalInput":
            if name != partition_name:
                in_names.append(name)
        elif alloc.kind == "ExternalOutput":
            out_names.append(name)
            out_avals.append(
                jax.core.ShapedArray(
                    tuple(alloc.tensor_shape), mybir.dt.np(alloc.dtype)
                )
            )
    assert in_names == ["comb", "wqk_aug", "bitm"], in_names
    assert out_names == ["outc"], out_names
    n_params = len(in_names)
    n_outs = len(out_names)
    all_names = list(in_names) + list(out_names)
    if partition_name is not None:
        all_names.append(partition_name)
    all_names = tuple(all_names)
    donate = tuple(range(n_params, n_params + n_outs))

    def _body(*args):
        operands = list(args)
        if partition_name is not None:
            operands.append(bass2jax.partition_id_tensor())
        outs = bass2jax._bass_exec_p.bind(
            *operands,
            out_avals=tuple(out_avals),
            in_names=all_names,
            out_names=tuple(out_names),
            lowering_input_output_aliases=(),
            sim_require_finite=True,
            sim_require_nnan=True,
            nc=nc,
        )
        return tuple(outs)

    devices = jax.devices()[:N_CORES]
    assert len(devices) == N_CORES
    mesh = Mesh(np.asarray(devices), ("core",))
    spec = PartitionSpec("core")
    ex = _Exec()
    ex.sharding = NamedSharding(mesh, spec)
    ex.sharded = jax.jit(
        shard_map(
            _body,
            mesh=mesh,
            in_specs=(spec,) * (n_params + n_outs),
            out_specs=(spec,) * n_outs,
            check_rep=False,
        ),
        donate_argnums=donate,
        keep_unused=True,
    )
    ex.zeros_fn = jax.jit(
        lambda: jnp.zeros((FS, N, V + 2), jnp.uint8), out_shardings=ex.sharding
    )
    bitmask = np.tile(np.array([0x80 >> k for k in range(8)], np.uint8), NB)
    ex.bitm_dev = jax.device_put(
        np.ascontiguousarray(np.broadcast_to(bitmask, (N_CORES * 128, N))),
        ex.sharding,
    )
    ex.device_put = jax.device_put
    ex.zeros_next = []
    return ex


def _get_exec():
    global _EXEC
    if _EXEC is None:
        _EXEC = _build_exec()
    return _EXEC


def _aug(W, b):
    aug = np.zeros((V + 1, QK), np.float32)
    aug[0:V] = np.asarray(W, np.float32).T
    aug[V] = np.asarray(b, np.float32)
    return aug


_BITW = np.array([128, 64, 32, 16, 8, 4, 2, 1], np.float32)

# Fused nogil encode/decode: the container has one CPU shared with the axon
# relay, so every host cycle saved is wire bandwidth gained.  numba versions
# are ~4x leaner than the numpy multi-pass path and bit-exact with it.
try:
    import numba

    @numba.njit(nogil=True, cache=True, fastmath=True)
    def _rowmax_nb(values, rm):
        m, n, v = values.shape
        for i in range(m):
            for r in range(n):
                am = 0.0
                for k in range(v):
                    a = abs(values[i, r, k])
                    if a > am:
                        am = a
                rm[i, r] = am

    @numba.njit(nogil=True, cache=True, fastmath=True)
    def _enc_nb(values, adj, comb_u8, comb_i8, sf):
        m, n, v = values.shape
        for i in range(m):
            for r in range(n):
                inv = 1.0 / sf[i, r]
                for k in range(v):
                    x = values[i, r, k] * inv
                    if x > 127.0:
                        x = 127.0
                    elif x < -127.0:
                        x = -127.0
                    comb_i8[i, r, k] = np.int8(round(x))
                for b in range(NB):
                    byte = 0
                    base = 8 * b
                    for t in range(8):
                        if adj[i, r, base + t] > 0.5:
                            byte |= 128 >> t
                    comb_u8[i, r, V + 2 + b] = byte

    @numba.njit(nogil=True, cache=True, fastmath=True)
    def _dec_nb(out8, osc, outf):
        m, n, _ = out8.shape
        for i in range(m):
            for r in range(n):
                s = osc[i, r]
                for k in range(V):
                    outf[i, 0, r, k] = (
                        np.float32(out8[i, r, k]) - np.float32(128.0)
                    ) * s

    _HAVE_NUMBA = True
except ImportError:  # pragma: no cover - numba is present in this container
    _HAVE_NUMBA = False


def kernel(**inputs):
    ex = _get_exec()
    values = np.asarray(inputs["values"], dtype=np.float32).reshape(F, N, V)
    adj = np.asarray(inputs["adjacency_matrix"], dtype=np.float32).reshape(F, N, N)

    wqk_dev = ex.device_put(
        np.tile(
            np.concatenate(
                [_aug(inputs["Wq"], inputs["bq"]), _aug(inputs["Wk"], inputs["bk"])]
            ),
            (N_CORES, 1),
        ),
        ex.sharding,
    )

    # host encode into the combined wire array: values -> int8 with per-row
    # abs-max scales (shipped /127 as f16 bytes), adjacency -> packed bits
    # via a BLAS matvec over the exact 0.0/1.0 floats (np.packbits is
    # GIL-bound, BLAS isn't).
    # staging buffer reused across calls: avoids ~77MB of first-touch page
    # faults per call (internal only -- every put completes before return)
    comb = getattr(ex, "comb_buf", None)
    if comb is None:
        comb = ex.comb_buf = np.empty((F, N, W), np.uint8)
    comb_i8 = comb.view(np.int8)

    if _HAVE_NUMBA:

        def _encode(a, b):
            rm = np.empty((b - a, N), np.float32)
            _rowmax_nb(values[a:b], rm)
            s16 = (rm * (1.0 / 127.0)).astype(np.float16)
            comb[a:b, :, V : V + 2] = s16[..., None].view(np.uint8)
            sf = np.maximum(s16.astype(np.float32), 1e-12)
            _enc_nb(values[a:b], adj[a:b], comb[a:b], comb_i8[a:b], sf)

    else:

        def _encode(a, b):
            v = values[a:b]
            rm = np.maximum(v.max(axis=-1), -v.min(axis=-1))
            s16 = (rm * (1.0 / 127.0)).astype(np.float16)
            comb[a:b, :, V : V + 2] = s16[..., None].view(np.uint8)
            sf = s16.astype(np.float32)
            np.maximum(sf, 1e-12, out=sf)
            np.reciprocal(sf, out=sf)
            t = v * sf[..., None]
            np.rint(t, out=t)
            np.clip(t, -127, 127, out=t)
            comb[a:b, :, 0:V].view(np.int8)[:] = t
            comb[a:b, :, V + 2 : W] = (adj[a:b].reshape(-1, 8) @ _BITW).reshape(
                b - a, N, NB
            )

    # segment pipeline over the full-duplex tunnel: the encode->dispatch
    # loop naturally paces uploads ~25ms apart (keeping 2-3 in flight --
    # the transport's sweet spot); each exec is dispatched immediately and
    # waits for its input device-side; fetches stream back through the
    # 2-deep pool, overlapping the remaining uploads.
    zeros = list(ex.zeros_next)
    while len(zeros) < SEG:
        zeros.append(ex.zeros_fn())
    ex.zeros_next = []
    outf = np.empty((F, 1, N, V), np.float32)
    outs = []
    for s in range(SEG):
        a, b = s * FS, (s + 1) * FS
        _encode(a, b)
        cf = _IO_POOL.submit(ex.device_put, comb[a:b], ex.sharding)
        z = zeros[s]
        if hasattr(z, "result"):
            z = z.result()
        out = ex.sharded(cf.result(), wqk_dev, ex.bitm_dev, z)[0]
        outs.append(_GET_POOL.submit(np.asarray, out))

    for s in range(SEG):
        a = s * FS
        out8 = outs[s].result()  # [FS, N, V+2] uint8, f16 scale embedded
        osc = (
            out8[:, :, V : V + 2]
            .copy()
            .view(np.float16)
            .reshape(FS, N)
            .astype(np.float32)
        )
        if _HAVE_NUMBA:
            _dec_nb(out8, osc, outf[a : a + FS])
        else:
            outf[a : a + FS, 0] = (
                out8[:, :, 0:V].astype(np.float32) - 128.0
            ) * osc[..., None]
    # donation buffers for the next call, created after the wire drains so
    # their RPCs don't steal relay CPU from this call's fetches
    ex.zeros_next = [_IO_POOL.submit(ex.zeros_fn) for _ in range(SEG)]
    return outf


# revision 50
# speedup vs baseline: 1.0657x; 1.0657x over previous
"""GNN message-passing attention kernel for Trainium2 (Bass/Tile).

Problem: 3 iterations of masked single-head attention over 1024 independent
graphs (N=256 nodes, V=40 features, QK=50), data-parallel on the leading F
axis across 8 NeuronCores (128 graphs/core), full inputs in / full output out.

The axon tunnel to the devices moves ~64 MB/s up / ~30 MB/s down with
~100 ms sync latency, so end-to-end time is dominated by host<->device
bytes and round trips, not device compute (~1 ms/core).  This version
minimizes wire traffic and pipelines it:
  - values cross the wire as int8 with per-row abs-max scales (10.5 MB +
    0.5 MB f16 scales) and are dequantized on-device; the ones-column used
    to fold the q/k biases into the matmuls is memset on-device.
  - adjacency crosses bit-packed (8.4 MB, the entropy floor for random 0/1)
    and is unpacked on the DVE: a broadcast-AP bitwise_and against a
    per-column bitmask, then is_gt(0) -> exact {0,1} bf16.  Host packing is
    a BLAS matvec over the exact 0.0/1.0 floats (np.packbits is GIL-bound).
  - the additive softmax mask is applied by PE matmuls with the unpacked
    adjacency as the *stationary* operand and a MASKC-scaled identity
    streaming, which wants adj[j,l] in its natural row-major layout -- no
    host-side transpose at all.  (MASKC rounds to 7072 in bf16; the +0.13
    shift after /sqrt(50) is uniform across unmasked entries of a row and
    cancels in softmax.)
  - the output is normalized then quantized on-device to uint8 with per-row
    abs-max scales: trunc(x*127/rowmax + 128.5) is exact round-to-nearest
    through the engine's trunc-toward-zero f32->u8 conversion, and the f16
    scale is embedded in the same row via a bitcast view (one 11 MB fetch).
  - all per-graph wire data rides ONE combined uint8 array per direction
    ([*, N, 74] in: int8 values | f16 scale | packed adj; [*, N, 42] out),
    one put + one fetch per segment.
  - the batch is split into SEG=8 segments pipelined over the full-duplex
    tunnel: host encode (fused numba loops -- the single CPU is shared with
    the axon relay) paces uploads so 2-3 stay in flight (the transport's
    sweet spot: deep async queues anti-scale to 18 MB/s, ~2 concurrent
    transfers sustain ~59 MB/s); execs dispatch immediately and wait for
    input device-side; fetches stream back through a 2-deep pool overlapping
    the remaining uploads; donated output buffers (jnp.zeros, device-side)
    for the next call are pre-created after the wire drains; the bitmask
    constant lives on-device across calls.
  - rel err vs the fp32 reference is ~1.1e-2 (int8 in + u8 out quantization,
    deterministic), within the 2e-2 gate.

Dataflow on-device (inherited from the previous version): "transposed-e"
layout, gb=2 graphs per pipeline step, 8 streams phase-interleaved so every
engine always has independent work queued.  e^T[l,j] = k_l . q_j accumulated
on top of the PE-written mask; one Exp ACT per pair produces num^T directly
in the layout the nv matmul wants; per-partition reciprocal + tensor_scalar
normalize during the PSUM->SBUF move, with rowsum*recip == 1.0 refreshing
the ones-column for the next iteration for free.
"""

import math
import sys
from concurrent.futures import ThreadPoolExecutor

import numpy as np

sys.path.insert(0, "/opt/trn_rl_repo")

import concourse.bass as bass  # noqa: E402,F401
import concourse.mybir as mybir  # noqa: E402
from concourse import bacc, bass2jax, tile  # noqa: E402
from concourse.masks import make_identity  # noqa: E402

# Problem constants (hardcoded per harness contract).
F, N, V, QK = 1024, 256, 40, 50
ITERS = 3
SCALE = math.sqrt(50.0)  # NUM_QK = 50
MASKC = 1000.0 * SCALE  # adj * MASKC accumulated into e; exp bias -1000
N_CORES = 8
SEG = 8  # upload/exec/download pipeline segments (the tunnel is full-duplex)
FS = F // SEG  # graphs per segment
G = FS // N_CORES  # graphs per core per segment
NC2 = N // 128  # 2 partition chunks of the node axis
NB = N // 8  # packed adjacency bytes per row
W = V + 2 + NB  # combined wire row: int8 values | f16 scale bytes | packed adj

F32 = mybir.dt.float32
F32R = mybir.dt.float32r  # fp32 data through the fast (replicated) PE path
BF16 = mybir.dt.bfloat16
F16 = mybir.dt.float16
U8 = mybir.dt.uint8
I8 = mybir.dt.int8

DEFAULT_BUFS = dict(
    io=10, work=10, small=11, vnb=22, vhb=8, adjpb=8, andb=8, vob=10,
    pmain=3, paux=2,
)


def build_nc(g_count=G, gb=2, streams=8, group=4, bufs=None):
    """Build the single-core Bass program (SPMD across 8 cores)."""
    B = dict(DEFAULT_BUFS)
    if bufs:
        B.update(bufs)
    streams = min(streams, g_count // gb)
    assert g_count % (gb * streams) == 0
    group = min(group, streams)
    nc = bacc.Bacc("TRN2", target_bir_lowering=False, debug=False)

    comb_d = nc.dram_tensor("comb", [g_count, N, W], U8, kind="ExternalInput")
    wqk_d = nc.dram_tensor("wqk_aug", [2 * (V + 1), QK], F32R, kind="ExternalInput")
    bitm_d = nc.dram_tensor("bitm", [128, N], U8, kind="ExternalInput")
    outc_d = nc.dram_tensor("outc", [g_count, N, V + 2], U8, kind="ExternalOutput")

    with tile.TileContext(nc) as tc:
        with (
            tc.tile_pool(name="const", bufs=1) as constp,
            tc.tile_pool(name="io", bufs=B["io"]) as iop,
            tc.tile_pool(name="work", bufs=B["work"]) as workp,
            tc.tile_pool(name="small", bufs=B["small"]) as smallp,
            tc.tile_pool(name="pmain", bufs=B["pmain"], space="PSUM") as pmainp,
            tc.tile_pool(name="paux", bufs=B["paux"], space="PSUM") as pauxp,
        ):
            wq_sb = constp.tile([V + 1, QK], F32R)
            nc.sync.dma_start(wq_sb, wqk_d[0 : V + 1, :])
            wk_sb = constp.tile([V + 1, QK], F32R)
            nc.sync.dma_start(wk_sb, wqk_d[V + 1 : 2 * (V + 1), :])
            bitm_sb = constp.tile([128, N], U8)
            nc.sync.dma_start(bitm_sb, bitm_d[:, :])
            expbias_sb = constp.tile([128, 1], F32)
            nc.gpsimd.memset(expbias_sb, -1000.0)
            id_f32 = constp.tile([128, 128], F32)
            make_identity(nc, id_f32)
            # MASKC-scaled identity: streamed against stationary adjacency
            # chunks to accumulate the additive mask into PSUM on PE.
            idm_sc = constp.tile([128, 128], BF16)
            nc.vector.tensor_scalar_mul(idm_sc, id_f32, MASKC)

            class Stream:
                pass

            def phase_load(st, g0):
                st.prev_g0 = getattr(st, "g0", None)
                st.prev_vo = getattr(st, "vo", None)
                st.g0 = g0
                gsl = slice(g0, g0 + gb)
                st.comb = iop.tile([128, gb, NC2, W], U8, tag="comb", bufs=B["adjpb"])
                nc.sync.dma_start(
                    st.comb, comb_d[gsl, :, :].rearrange("g (c p) w -> p g c w", c=NC2)
                )

            def phase_prep(st):
                # int8 -> fp32 dequant by the per-row scale (already /127 on
                # host); the ones-column rides the same tile so the q/k
                # biases stay inside the weight matmuls.
                vsc = smallp.tile([128, gb, NC2], F32, tag="vsc")
                nc.vector.tensor_copy(
                    vsc.unsqueeze(-1), st.comb[:, :, :, V : V + 2].bitcast(F16)
                )
                st.vn = iop.tile([128, gb, NC2, V + 1], F32, tag="vn", bufs=B["vnb"])
                for g in range(gb):
                    for c in range(NC2):
                        nc.vector.tensor_scalar_mul(
                            st.vn[:, g, c, 0:V],
                            st.comb[:, g, c, 0:V].bitcast(I8),
                            vsc[:, g, c : c + 1],
                        )
                nc.gpsimd.memset(st.vn[:, :, :, V], 1.0)
                # unpack adjacency bits: (byte & bitmask) > 0 -> {0,1} bf16,
                # laid out adj[j-part, l-free] for stationary mask matmuls.
                t_and = smallp.tile([128, gb, NC2, N], U8, tag="andt", bufs=B["andb"])
                src = (
                    st.comb[:, :, :, V + 2 : W]
                    .rearrange("p g c b -> p (g c) b")
                    .unsqueeze(-1)
                    .broadcast_to([128, gb * NC2, NB, 8])
                )
                msk = (
                    bitm_sb[:, :]
                    .rearrange("p (b e) -> p b e", e=8)
                    .unsqueeze(1)
                    .broadcast_to([128, gb * NC2, NB, 8])
                )
                dst = t_and[:, :, :, :].rearrange("p g c (b e) -> p (g c) b e", e=8)
                nc.vector.tensor_tensor(dst, src, msk, op=mybir.AluOpType.bitwise_and)
                st.adj = iop.tile([128, gb, NC2, N], BF16, tag="adj")
                nc.vector.tensor_single_scalar(
                    st.adj, t_and, 0, op=mybir.AluOpType.is_gt
                )
                st.comb = None

            def phase_vt0(st):
                psum_vt = pauxp.tile([V + 1, gb * N], F32, tag="paux")
                for g in range(gb):
                    for c in range(NC2):
                        nc.tensor.transpose(
                            psum_vt[:, N * g + 128 * c : N * g + 128 * (c + 1)],
                            st.vn[:, g, c, :],
                            id_f32,
                        )
                st.vt = smallp.tile([V + 1, gb * N], F32R, tag="vt")
                nc.vector.tensor_copy(st.vt, psum_vt)

            def phase_qk(st):
                # [50, (qk-half, g, j)]: q in bank 0, k in bank 1.
                # Bias rides the vt ones-row (weights row V).
                st.psum_qk = pmainp.tile([QK, 2 * gb * N], F32, tag="pmain")
                nc.tensor.matmul(st.psum_qk[:, 0 : gb * N], wq_sb, st.vt)
                nc.tensor.matmul(st.psum_qk[:, gb * N : 2 * gb * N], wk_sb, st.vt)

            def phase_tanh(st):
                st.qk = workp.tile([QK, 2 * gb * N], F32R, tag="qk")
                nc.scalar.activation(
                    st.qk, st.psum_qk, mybir.ActivationFunctionType.Tanh
                )
                st.psum_qk = None

            def phase_mask(st):
                # additive mask preloaded into PSUM on PE: stationary
                # adjacency chunk [j-part, l-free], streaming MASKC-scaled
                # identity -> psum_e[l, j] = MASKC * adj[j, l].
                st.psum_e = pmainp.tile([128, gb, NC2 * N], F32, tag="pmain", name="pe")
                # each graph's e-block is one 2KB PSUM zero region; start=True
                # (which re-marks the whole region pending-zero) only on the
                # first of its four chunk matmuls -- the rest land on
                # still-pending bytes and overwrite their own chunk.
                for g in range(gb):
                    for lc in range(NC2):
                        for jc in range(NC2):
                            nc.tensor.matmul(
                                st.psum_e[
                                    :, g, N * lc + 128 * jc : N * lc + 128 * (jc + 1)
                                ],
                                st.adj[:, g, jc, 128 * lc : 128 * (lc + 1)],
                                idm_sc,
                                start=(lc == 0 and jc == 0),
                                stop=False,
                                skip_group_check=True,
                            )

            def phase_et(st):
                for g in range(gb):
                    for lc in range(NC2):
                        nc.tensor.matmul(
                            st.psum_e[:, g, N * lc : N * (lc + 1)],
                            st.qk[:, gb * N + N * g + 128 * lc : gb * N + N * g + 128 * (lc + 1)],
                            st.qk[:, N * g : N * (g + 1)],
                            start=False,
                            stop=True,
                            skip_group_check=True,
                        )

            def phase_exp(st):
                st.numt = workp.tile([128, gb, NC2 * N], F32, tag="numt")
                nc.scalar.activation(
                    st.numt,
                    st.psum_e,
                    mybir.ActivationFunctionType.Exp,
                    bias=expbias_sb,
                    scale=1.0 / SCALE,
                )
                st.psum_e = None

            def phase_nv(st):
                # nv[j, v] = sum_l num[j, l] v[l, v], directly off numT
                # (l already on partitions); the vn ones-column makes col V
                # the softmax row-sum.
                st.psum_nv = pauxp.tile([128, gb, NC2, V + 1], F32, tag="paux")
                for g in range(gb):
                    for jc in range(NC2):
                        for lc in range(NC2):
                            nc.tensor.matmul(
                                st.psum_nv[:, g, jc, :],
                                st.numt[:, g, N * lc + 128 * jc : N * lc + 128 * jc + 128],
                                st.vn[:, g, lc, :],
                                start=(lc == 0),
                                stop=(lc == NC2 - 1),
                            )
                st.numt = None

            def phase_norm(st, last):
                recip = smallp.tile([128, gb, NC2], F32, tag="recip")
                nc.vector.reciprocal(recip, st.psum_nv[:, :, :, V])
                if last:
                    # final iteration: normalize, then quantize each row to
                    # uint8 with its own abs-max scale.  trunc(x*127/rowmax
                    # + 128.5) is exact round-to-nearest (everything
                    # positive, so the engine's trunc-toward-zero == floor;
                    # max lands on 255.5-eps, no wrap); host decodes as
                    # (k - 128) * (rowmax/127) from the shipped fp16 scale.
                    vo32 = workp.tile([128, gb, NC2, V], F32, tag="vo32")
                    rowmax = smallp.tile([128, gb, NC2], F32, tag="rowmax")
                    for g in range(gb):
                        for jc in range(NC2):
                            nc.vector.tensor_scalar_mul(
                                vo32[:, g, jc, :],
                                st.psum_nv[:, g, jc, 0:V],
                                recip[:, g, jc : jc + 1],
                            )
                            nc.vector.tensor_reduce(
                                rowmax[:, g, jc : jc + 1],
                                vo32[:, g, jc, :],
                                axis=mybir.AxisListType.X,
                                op=mybir.AluOpType.max,
                                apply_absolute_value=True,
                            )
                    qs = smallp.tile([128, gb, NC2], F32, tag="qs")
                    nc.vector.reciprocal(qs, rowmax)
                    qs127 = smallp.tile([128, gb, NC2], F32, tag="qs127")
                    nc.vector.tensor_scalar_mul(qs127, qs, 127.0)
                    st.vo = workp.tile(
                        [128, gb, NC2, V + 2], U8, tag="vo", bufs=B["vob"]
                    )
                    nc.vector.tensor_scalar_mul(
                        st.vo[:, :, :, V : V + 2].bitcast(F16),
                        rowmax.unsqueeze(-1),
                        1.0 / 127.0,
                    )
                    for g in range(gb):
                        for jc in range(NC2):
                            nc.vector.tensor_scalar(
                                st.vo[:, g, jc, 0:V],
                                vo32[:, g, jc, :],
                                qs127[:, g, jc : jc + 1],
                                128.5,
                                op0=mybir.AluOpType.mult,
                                op1=mybir.AluOpType.add,
                            )
                else:
                    st.vn = iop.tile([128, gb, NC2, V + 1], F32, tag="vn", bufs=B["vnb"])
                    for g in range(gb):
                        for jc in range(NC2):
                            nc.vector.tensor_scalar_mul(
                                st.vn[:, g, jc, :],
                                st.psum_nv[:, g, jc, :],
                                recip[:, g, jc : jc + 1],
                            )
                st.psum_nv = None

            def phase_vt(st):
                psum_vt = pauxp.tile([V + 1, gb * N], F32, tag="paux")
                for g in range(gb):
                    for jc in range(NC2):
                        nc.tensor.transpose(
                            psum_vt[:, N * g + 128 * jc : N * g + 128 * (jc + 1)],
                            st.vn[:, g, jc, :],
                            id_f32,
                        )
                st.vt = smallp.tile([V + 1, gb * N], F32R, tag="vt")
                nc.vector.tensor_copy(st.vt, psum_vt)

            def phase_store_prev(st):
                # SWDGE (gpsimd) queue: keeps result stores out of the SP
                # FIFO so the next round's loads always prefetch early.
                gsl = slice(st.prev_g0, st.prev_g0 + gb)
                nc.gpsimd.dma_start(
                    outc_d[gsl, :, :].rearrange("g (c p) v -> p g c v", c=NC2),
                    st.prev_vo,
                )

            sts = [Stream() for _ in range(streams)]
            for _i, _st in enumerate(sts):
                _st.sid = _i
            grps = [sts[i : i + group] for i in range(0, streams, group)]

            def run_iter(grp, t):
                for st in grp:
                    phase_qk(st)
                for st in grp:
                    phase_mask(st)
                for st in grp:
                    phase_tanh(st)
                for st in grp:
                    phase_et(st)
                for st in grp:
                    phase_exp(st)
                for st in grp:
                    phase_nv(st)
                for st in grp:
                    phase_norm(st, t == ITERS - 1)
                if t < ITERS - 1:
                    for st in grp:
                        phase_vt(st)

            # Groups round-robin per iteration so one group's next phase
            # fills the pipeline while the other finishes; the previous
            # round's store and the next round's load ride inside the
            # rotation so round boundaries never resynchronize the streams.
            rounds = g_count // (gb * streams)
            for r in range(rounds):
                for grp in grps:
                    for st in grp:
                        phase_load(st, gb * (r * streams + st.sid))
                for grp in grps:
                    for st in grp:
                        if r > 0:
                            phase_store_prev(st)
                    for st in grp:
                        phase_prep(st)
                    for st in grp:
                        phase_vt0(st)
                for t in range(ITERS):
                    for grp in grps:
                        run_iter(grp, t)
            for grp in grps:
                for st in grp:
                    st.prev_g0, st.prev_vo = st.g0, st.vo
                    phase_store_prev(st)

    nc.compile()
    return nc


# ---------------------------------------------------------------------------
# Execution path: cached jitted shard_map over 8 cores, bypassing
# run_bass_via_pjrt's host-side concats / host-zero donation buffers.
# ---------------------------------------------------------------------------

_IO_POOL = ThreadPoolExecutor(32)  # wire gets + zeros (block, don't compute)
# 2-deep fetch pool: bounded downlink concurrency avoids the deep-queue
# anti-scaling the transport shows when many transfers are registered at once
_GET_POOL = ThreadPoolExecutor(2)


class _Exec:
    pass


_EXEC = None


def _build_exec():
    import jax
    import jax.numpy as jnp
    from jax.experimental.shard_map import shard_map
    from jax.sharding import Mesh, NamedSharding, PartitionSpec

    nc = build_nc()
    bass2jax.install_neuronx_cc_hook()
    assert nc.dbg_addr is None
    partition_name = nc.partition_id_tensor.name if nc.partition_id_tensor else None

    in_names, out_names, out_avals = [], [], []
    for alloc in nc.m.functions[0].allocations:
        if not isinstance(alloc, mybir.MemoryLocationSet):
            continue
        name = alloc.memorylocations[0].name
        if alloc.kind == "ExternalInput":
            if name != partition_name:
                in_names.append(name)
        elif alloc.kind == "ExternalOutput":
            out_names.append(name)
            out_avals.append(
                jax.core.ShapedArray(
                    tuple(alloc.tensor_shape), mybir.dt.np(alloc.dtype)
                )
            )
    assert in_names == ["comb", "wqk_aug", "bitm"], in_names
    assert out_names == ["outc"], out_names
    n_params = len(in_names)
    n_outs = len(out_names)
    all_names = list(in_names) + list(out_names)
    if partition_name is not None:
        all_names.append(partition_name)
    all_names = tuple(all_names)
    donate = tuple(range(n_params, n_params + n_outs))

    def _body(*args):
        operands = list(args)
        if partition_name is not None:
            operands.append(bass2jax.partition_id_tensor())
        outs = bass2jax._bass_exec_p.bind(
            *operands,
            out_avals=tuple(out_avals),
            in_names=all_names,
            out_names=tuple(out_names),
            lowering_input_output_aliases=(),
            sim_require_finite=True,
            sim_require_nnan=True,
            nc=nc,
        )
        return tuple(outs)

    devices = jax.devices()[:N_CORES]
    assert len(devices) == N_CORES
    mesh = Mesh(np.asarray(devices), ("core",))
    spec = PartitionSpec("core")
    ex = _Exec()
    ex.sharding = NamedSharding(mesh, spec)
    ex.sharded = jax.jit(
        shard_map(
            _body,
            mesh=mesh,
            in_specs=(spec,) * (n_params + n_outs),
            out_specs=(spec,) * n_outs,
            check_rep=False,
        ),
        donate_argnums=donate,
        keep_unused=True,
    )
    ex.zeros_fn = jax.jit(
        lambda: jnp.zeros((FS, N, V + 2), jnp.uint8), out_shardings=ex.sharding
    )
    bitmask = np.tile(np.array([0x80 >> k for k in range(8)], np.uint8), NB)
    ex.bitm_dev = jax.device_put(
        np.ascontiguousarray(np.broadcast_to(bitmask, (N_CORES * 128, N))),
        ex.sharding,
    )
    ex.device_put = jax.device_put
    ex.block = jax.block_until_ready
    ex.zeros_next = []
    return ex


def _get_exec():
    global _EXEC
    if _EXEC is None:
        _EXEC = _build_exec()
    return _EXEC


def _aug(W, b):
    aug = np.zeros((V + 1, QK), np.float32)
    aug[0:V] = np.asarray(W, np.float32).T
    aug[V] = np.asarray(b, np.float32)
    return aug


_BITW = np.array([128, 64, 32, 16, 8, 4, 2, 1], np.float32)

# Fused nogil encode/decode: the container has one CPU shared with the axon
# relay, so every host cycle saved is wire bandwidth gained.  numba versions
# are ~4x leaner than the numpy multi-pass path and bit-exact with it.
try:
    import numba

    @numba.njit(nogil=True, cache=True, fastmath=True)
    def _rowmax_nb(values, rm):
        m, n, v = values.shape
        for i in range(m):
            for r in range(n):
                am = 0.0
                for k in range(v):
                    a = abs(values[i, r, k])
                    if a > am:
                        am = a
                rm[i, r] = am

    @numba.njit(nogil=True, cache=True, fastmath=True)
    def _enc_nb(values, adj, comb_u8, comb_i8, sf):
        m, n, v = values.shape
        for i in range(m):
            for r in range(n):
                inv = 1.0 / sf[i, r]
                for k in range(v):
                    x = values[i, r, k] * inv
                    if x > 127.0:
                        x = 127.0
                    elif x < -127.0:
                        x = -127.0
                    comb_i8[i, r, k] = np.int8(round(x))
                for b in range(NB):
                    byte = 0
                    base = 8 * b
                    for t in range(8):
                        if adj[i, r, base + t] > 0.5:
                            byte |= 128 >> t
                    comb_u8[i, r, V + 2 + b] = byte

    @numba.njit(nogil=True, cache=True, fastmath=True)
    def _dec_nb(out8, osc, outf):
        m, n, _ = out8.shape
        for i in range(m):
            for r in range(n):
                s = osc[i, r]
                for k in range(V):
                    outf[i, 0, r, k] = (
                        np.float32(out8[i, r, k]) - np.float32(128.0)
                    ) * s

    _HAVE_NUMBA = True
except ImportError:  # pragma: no cover - numba is present in this container
    _HAVE_NUMBA = False


def kernel(**inputs):
    ex = _get_exec()
    values = np.asarray(inputs["values"], dtype=np.float32).reshape(F, N, V)
    adj = np.asarray(inputs["adjacency_matrix"], dtype=np.float32).reshape(F, N, N)

    wqk_dev = ex.device_put(
        np.tile(
            np.concatenate(
                [_aug(inputs["Wq"], inputs["bq"]), _aug(inputs["Wk"], inputs["bk"])]
            ),
            (N_CORES, 1),
        ),
        ex.sharding,
    )

    # host encode into the combined wire array: values -> int8 with per-row
    # abs-max scales (shipped /127 as f16 bytes), adjacency -> packed bits
    # via a BLAS matvec over the exact 0.0/1.0 floats (np.packbits is
    # GIL-bound, BLAS isn't).
    # staging buffer reused across calls: avoids ~77MB of first-touch page
    # faults per call (internal only -- every put completes before return)
    comb = getattr(ex, "comb_buf", None)
    if comb is None:
        comb = ex.comb_buf = np.empty((F, N, W), np.uint8)
    comb_i8 = comb.view(np.int8)

    if _HAVE_NUMBA:

        def _encode(a, b):
            rm = np.empty((b - a, N), np.float32)
            _rowmax_nb(values[a:b], rm)
            s16 = (rm * (1.0 / 127.0)).astype(np.float16)
            comb[a:b, :, V : V + 2] = s16[..., None].view(np.uint8)
            sf = np.maximum(s16.astype(np.float32), 1e-12)
            _enc_nb(values[a:b], adj[a:b], comb[a:b], comb_i8[a:b], sf)

    else:

        def _encode(a, b):
            v = values[a:b]
            rm = np.maximum(v.max(axis=-1), -v.min(axis=-1))
            s16 = (rm * (1.0 / 127.0)).astype(np.float16)
            comb[a:b, :, V : V + 2] = s16[..., None].view(np.uint8)
            sf = s16.astype(np.float32)
            np.maximum(sf, 1e-12, out=sf)
            np.reciprocal(sf, out=sf)
            t = v * sf[..., None]
            np.rint(t, out=t)
            np.clip(t, -127, 127, out=t)
            comb[a:b, :, 0:V].view(np.int8)[:] = t
            comb[a:b, :, V + 2 : W] = (adj[a:b].reshape(-1, 8) @ _BITW).reshape(
                b - a, N, NB
            )

    # segment pipeline over the full-duplex tunnel: the encode->dispatch
    # loop naturally paces uploads ~25ms apart (keeping 2-3 in flight --
    # the transport's sweet spot); each exec is dispatched immediately and
    # waits for its input device-side; fetches stream back through the
    # 2-deep pool, overlapping the remaining uploads.
    zeros = list(ex.zeros_next)
    while len(zeros) < SEG:
        zeros.append(ex.zeros_fn())
    ex.zeros_next = []
    outf = np.empty((F, 1, N, V), np.float32)
    outs = []
    put_watch = []
    for s in range(SEG):
        a, b = s * FS, (s + 1) * FS
        _encode(a, b)
        # hard cap of 3 in-flight uploads: free when the wire is fast (the
        # waited-on put is long done), prevents the deep-queue anti-scaling
        # regime from compounding when the wire degrades
        if s >= 3:
            put_watch[s - 3].result()
        cf = _IO_POOL.submit(ex.device_put, comb[a:b], ex.sharding)
        z = zeros[s]
        if hasattr(z, "result"):
            z = z.result()
        arr = cf.result()
        put_watch.append(_IO_POOL.submit(ex.block, arr))
        out = ex.sharded(arr, wqk_dev, ex.bitm_dev, z)[0]
        outs.append(_GET_POOL.submit(np.asarray, out))

    for s in range(SEG):
        a = s * FS
        out8 = outs[s].result()  # [FS, N, V+2] uint8, f16 scale embedded
        osc = (
            out8[:, :, V : V + 2]
            .copy()
            .view(np.float16)
            .reshape(FS, N)
            .astype(np.float32)
        )
        if _HAVE_NUMBA:
            _dec_nb(out8, osc, outf[a : a + FS])
        else:
            outf[a : a + FS, 0] = (
                out8[:, :, 0:V].astype(np.float32) - 128.0
            ) * osc[..., None]
    # donation buffers for the next call, created after the wire drains so
    # their RPCs don't steal relay CPU from this call's fetches
    ex.zeros_next = [_IO_POOL.submit(ex.zeros_fn) for _ in range(SEG)]
    return outf
